# revision 62
# baseline (speedup 1.0000x reference)
"""Linear-attention MultiHeadAttentionBlock kernel for 8 Trainium2 NeuronCores.

Sharding: core c handles (batch b = c//2, head-group g = c%2).  Each core
computes, for its batch's q/k/v and its 8 heads (512 of the 1024 d_model
dims):
    QfT = elu(Wq_g @ X_q^T)+1          (transposed space: d' on partitions)
    Kf  = elu(X_k @ Wk_g^T)+1          (n-space)
    Vp  = X_v @ Wv_g^T                 (n-space)
    KVT[c] = sum_n Vp_pair^T Kf_pair   (block-diag 128x128 per head pair)
    Zpre = Qf . ksum  -> zr = 1/Zpre   (eps negligible: Zpre ~ 1e4 > 0)
    QfT *= zr                          (broadcast via sel8 matmul)
    M[c] = blockdiag(KVT[c]) @ Wo_rows ([512 d', 1024] folded projection)
    y    = QfT^T @ M                   (n-space, accumulated over 4 chunks)
Host sums the two per-batch partials (y is already [L, d_model]).

All matmul operands are bfloat16 (host-side conversion; fp32 PSUM
accumulation).  MM_DTYPE=float32r/float32 env selects wider modes.

DMA strategy: few large DMAs (descriptor-generation is ~625ns serial per
DMA regardless of size): x as 8 full [128,2048] chunks per tensor,
weights repacked host-side to kc-major [128,4096] single transfers.
The Q projection runs kc-outer across 8 PSUM banks so each arriving x
chunk is consumed with ~1.7us of PE work and the PE never waits on DMA.
"""

import os

import numpy as np

import concourse.bass as bass
import concourse.mybir as mybir
import concourse.tile as tile
from concourse import bacc
from concourse.bass_utils import run_bass_kernel_spmd
from concourse.masks import make_identity

P = 128
L = 2048          # sequence length
DM = 1024         # d_model (= contraction dim of projections)
DG = 512          # per-core head-group width (8 heads x 64)
NT = L // P       # 16 n-tiles
KC = DM // P      # 8 contraction chunks
DT = DG // P      # 4 d'-tiles (2 heads each)
NCH = 4           # n-chunks of 512 for transposed-Q projection
F32 = mybir.dt.float32

_CACHE = {}


def _mm_dtype():
    name = os.environ.get("MM_DTYPE", "bfloat16")
    return {"bfloat16": mybir.dt.bfloat16,
            "float32r": mybir.dt.float32r,
            "float32": F32}[name]


def build_nc(repeats=1):
    MMDT = _mm_dtype()
    nc = bacc.Bacc(None, target_bir_lowering=False)

    xq_d = nc.dram_tensor("xqT", [DM, L], MMDT, kind="ExternalInput")
    xk_d = nc.dram_tensor("xkT", [DM, L], MMDT, kind="ExternalInput")
    xv_d = nc.dram_tensor("xvT", [DM, L], MMDT, kind="ExternalInput")
    # weights kc-major: [128, kc*512 + d'] etc.
    wq_d = nc.dram_tensor("wqC", [P, KC * DG], MMDT, kind="ExternalInput")
    wk_d = nc.dram_tensor("wkC", [P, KC * DG], MMDT, kind="ExternalInput")
    wv_d = nc.dram_tensor("wvC", [P, KC * DG], MMDT, kind="ExternalInput")
    # w_o chunk-major: [128, c*1024 + dm]
    wo_d = nc.dram_tensor("woC", [P, DT * DM], MMDT, kind="ExternalInput")
    sel_d = nc.dram_tensor("sel8", [8, DT * P], MMDT, kind="ExternalInput")
    # y partials in bf16: host sums in fp32.  Halves the store traffic and
    # doubles DVE copy rate; adds ~1e-3 relative error (budget is 2e-2).
    y_d = nc.dram_tensor("y", [L, DM], MMDT, kind="ExternalOutput")

    with tile.TileContext(nc) as tc:
        with (
            tc.tile_pool(name="const", bufs=1) as cpool,
            tc.tile_pool(name="xt", bufs=18) as xt,      # (128,2048) full chunks
            tc.tile_pool(name="wt", bufs=3) as wt,       # (128,4096) weight cats
            tc.tile_pool(name="wo", bufs=1) as wop,      # (128,4096) w_o cat
            tc.tile_pool(name="qft", bufs=16) as qftp,   # QfT persistent
            tc.tile_pool(name="kf", bufs=16) as kfp,     # Kf persistent
            tc.tile_pool(name="vp", bufs=3) as vpp,      # Vp rotating
            tc.tile_pool(name="tmp", bufs=4) as tmp,     # feature-map temps
            tc.tile_pool(name="kvsb", bufs=1) as kvsb,   # ksum2/zr/kvc2
            tc.tile_pool(name="msb", bufs=4) as msb,     # M chunks (128,1024)
            tc.tile_pool(name="ysb", bufs=4) as ysb,     # (128,1024) y row-tiles
            tc.tile_pool(name="pp", bufs=4, space="PSUM") as pp,    # (128,512)
            tc.tile_pool(name="kvp", bufs=4, space="PSUM") as kvp,  # accumulators
        ):
            for _rep in range(repeats):
                body(nc, tc, MMDT, cpool,
                     xt, wt, wop, qftp, kfp, vpp, tmp, kvsb, msb, ysb, pp, kvp,
                     xq_d, xk_d, xv_d, wq_d, wk_d, wv_d, wo_d, sel_d, y_d,
                     first=(_rep == 0))

    nc.compile()
    return nc


def body(nc, tc, MMDT, cpool,
         xt, wt, wop, qftp, kfp, vpp, tmp, kvsb, msb, ysb, pp, kvp,
         xq_d, xk_d, xv_d, wq_d, wk_d, wv_d, wo_d, sel_d, y_d, first=True):
    Exp = mybir.ActivationFunctionType.Exp
    Relu = mybir.ActivationFunctionType.Relu
    Alu = mybir.AluOpType

    # ---------------- Phase Q DMAs first (startup critical path) -------
    # wq kc0-1 and the first xq chunk lead in halves; everything else
    # follows as full-size transfers.
    wq_t = wt.tile([P, KC * DG], MMDT, tag="wt", name="wq_t")
    nc.sync.dma_start(wq_t[:, 0:2 * DG], wq_d[:, 0:2 * DG])
    xq_c = []
    for kc in range(KC):
        t = xt.tile([P, L], MMDT, tag="xt", name="xh")
        if kc == 0:
            nc.sync.dma_start(t[:, 0:1024], xq_d[0:P, 0:1024])
            nc.sync.dma_start(t[:, 1024:2048], xq_d[0:P, 1024:2048])
        else:
            nc.sync.dma_start(t[:], xq_d[kc * P:(kc + 1) * P, :])
        xq_c.append(t)
        if 1 <= kc <= 6:
            # wq slice kc+1 rides just ahead of its x chunk: every chunk's
            # transfer stays un-gated and the bubbles vanish
            nc.sync.dma_start(wq_t[:, (kc + 1) * DG:(kc + 2) * DG],
                              wq_d[:, (kc + 1) * DG:(kc + 2) * DG])

    # constants (engine-local setup; does not delay the SP DMA queue)
    if first:
        ident_b = cpool.tile([P, P], MMDT, name="ident_b")
        make_identity(nc, ident_b[:])
        ones = cpool.tile([P, 2], MMDT, name="ones")
        ones_f = cpool.tile([P, 2], F32, name="ones_f")
        nc.gpsimd.memset(ones_f[:], 1.0)
        nc.vector.tensor_copy(ones[:], ones_f[:])
        _CACHE["_const"] = (ones, ident_b)
    else:
        ones, ident_b = _CACHE["_const"]

    if first:
        # p-state warmup while the PE waits for the first DMAs: transposes
        # across 4 rotating PSUM tiles pipeline back-to-back (a single tile
        # would serialize on WAW), so ~3us of continuous busy brings the PE
        # to full clock before the first real matmul.
        warm = [kvp.tile([P, P], MMDT, tag="acc", name=f"warm{_i}")
                for _i in range(4)]
        for i in range(24):
            nc.tensor.transpose(warm[i % 4][:], ident_b[:], ident_b[:])



    def wslice(w_t, kc, dt=None):
        if dt is None:
            return w_t[:, kc * DG:(kc + 1) * DG]
        return w_t[:, kc * DG + dt * P:kc * DG + (dt + 1) * P]

    def feature_map(ps, dst, alt=False):
        # dst = elu(ps)+1 = exp(min(ps,0)) + relu(ps)
        t0 = tmp.tile([P, 512], F32, tag="t0", bufs=12, name="t0")
        t1 = tmp.tile([P, 512], MMDT, tag="t1", bufs=14, name="t1")
        nc.vector.tensor_scalar(t0[:], ps[:], 0.0, None, Alu.min)
        if alt:
            # keep the saturated ACT queue short: relu on DVE
            nc.vector.tensor_scalar(t1[:], ps[:], 0.0, None, Alu.max)
        else:
            nc.scalar.activation(t1[:], ps[:], Relu)
        nc.scalar.activation(dst[:], t0[:], Exp)
        nc.gpsimd.tensor_tensor(dst[:], dst[:], t1[:], Alu.add)

    # Quick-release variant: one copy (PSUM -> bf16 dst) frees the PSUM
    # bank in ~0.6us; the elu math runs in place later (bf16 DVE ops are
    # 2x rate), interleaved into the K phase.  Returns the deferred stage.
    def fmap_release(ps, dst, engine):
        engine(dst[:], ps[:])

        def finish():
            t1 = tmp.tile([P, 512], MMDT, tag="t1", bufs=14, name="t1")
            nc.vector.tensor_scalar(t1[:], dst[:], 0.0, None, Alu.max)
            nc.vector.tensor_scalar(dst[:], dst[:], 0.0, None, Alu.min)
            nc.scalar.activation(dst[:], dst[:], Exp)
            nc.vector.tensor_tensor(dst[:], dst[:], t1[:], Alu.add)
        return finish

    # ---------------- Phase Q: QfT (transposed) ----------------
    # half 0 runs kc-outer across 8 PSUM banks: each arriving xq chunk is
    # fully consumed (8 matmuls) before the next is needed, so the PE is
    # never DMA-paced.  half 1 (all chunks resident by then) runs dt-outer
    # two banks at a time so its feature maps drain during the half-1
    # matmuls and the K phase's PSUM ring is free the moment Q ends.
    qft = [None] * 16  # (128, 512) tiles: index dt*4 + half*2 + win
    psb = []
    for dt in range(DT):
        pool, tag = (pp, "pp") if dt < 2 else (kvp, "acc")
        psb.append([pool.tile([P, 512], F32, tag=tag, name=f"q{dt}")
                    for _w in range(2)])
    for kc in range(KC):
        for dt in range(DT):
            for win in range(2):
                nc.tensor.matmul(
                    psb[dt][win][:],
                    wslice(wq_t, kc, dt),
                    xq_c[kc][:, win * 512:(win + 1) * 512],
                    start=(kc == 0), stop=(kc == KC - 1),
                )
    # quick-release copies first (alternating ACT/DVE) so the PSUM rings
    # free at copy rate; the elu math drains during the K phase.
    finishers = []
    for dt in range(DT):
        for win in range(2):
            qf = qftp.tile([P, 512], MMDT, tag="qft")
            eng = nc.scalar.copy if (dt * 2 + win) % 2 == 0 \
                else nc.vector.tensor_copy
            finishers.append(fmap_release(psb[dt][win], qf, eng))
            qft[dt * NCH + win] = qf
    for dt in range(DT):
        ps0 = pp.tile([P, 512], F32, tag="pp", name="ps0")
        ps1 = pp.tile([P, 512], F32, tag="pp", name="ps1")
        for kc in range(KC):
            nc.tensor.matmul(
                ps0[:], wslice(wq_t, kc, dt), xq_c[kc][:, 1024:1536],
                start=(kc == 0), stop=(kc == KC - 1),
            )
            nc.tensor.matmul(
                ps1[:], wslice(wq_t, kc, dt), xq_c[kc][:, 1536:2048],
                start=(kc == 0), stop=(kc == KC - 1),
            )
        for win, ps in ((0, ps0), (1, ps1)):
            qf = qftp.tile([P, 512], MMDT, tag="qft")
            eng = nc.scalar.copy if win == 0 else nc.vector.tensor_copy
            finishers.append(fmap_release(ps, qf, eng))
            qft[dt * NCH + 2 + win] = qf

    def qft_block(dt, nt):
        # (128, 128) slice: QfT[dt-block][:, nt*128 : nt*128+128]
        t = qft[dt * NCH + (nt * P) // 512]
        off = (nt * P) % 512
        return t[:, off:off + P]

    # ---------------- Phase K: Kf (n-space) ----------------
    wk_t = wt.tile([P, KC * DG], MMDT, tag="wt", name="wk_t")
    nc.sync.dma_start(wk_t[:], wk_d[:, :])
    xk_c = []
    for kc in range(KC):
        t = xt.tile([P, L], MMDT, tag="xt", name="xh")
        nc.sync.dma_start(t[:], xk_d[kc * P:(kc + 1) * P, :])
        xk_c.append(t)
    sel_t = cpool.tile([8, DT * P], MMDT, name="sel_t")
    nc.sync.dma_start(sel_t[:], sel_d[:, :])

    kf = []
    kparts = []
    ksumb = [kvp.tile([P, 2], F32, tag="acc", name=f"ksumb{_d}")
             for _d in range(DT)]

    def ksum_mm(nt):
        for dt in range(DT):
            nc.tensor.matmul(
                ksumb[dt][:],
                kf[nt][:, dt * P:(dt + 1) * P],
                ones[:],
                start=(nt == 0), stop=(nt == NT - 1),
            )

    for nt in range(NT):
        ps = pp.tile([P, 512], F32, tag="pp")
        for kc in range(KC):
            nc.tensor.matmul(
                ps[:],
                xk_c[kc][:, nt * P:(nt + 1) * P],
                wslice(wk_t, kc),
                start=(kc == 0), stop=(kc == KC - 1),
            )
        kft = kfp.tile([P, 512], MMDT, tag="kf")
        feature_map(ps, kft, alt=(nt % 2 == 1))
        kf.append(kft)
        # one deferred Q elu per K tile: spreads the SBUF-side work over
        # the whole phase on the 2x-rate bf16 DVE path
        finishers[nt]()
        # ksum for nt-3: three tiles of slack so PE never waits on the
        # ~3us feature-map chain even with ACT queue backlog
        if nt > 3:
            ksum_mm(nt - 4)
    for nt_ in (NT - 4, NT - 3, NT - 2, NT - 1):
        ksum_mm(nt_)

    # ksum2 columns from ksumb (frees the kvp ring for the kvt accumulators)
    ksum2 = kvsb.tile([P, 2 * DT], MMDT, tag="ksum2")
    for dt in range(DT):
        nc.scalar.copy(ksum2[0:64, 2 * dt:2 * dt + 1],
                       ksumb[dt][0:64, 0:1])
        nc.scalar.mul(ksum2[64:128, 2 * dt:2 * dt + 1],
                      ksumb[dt][64:128, 0:1], 0.0)
        nc.scalar.mul(ksum2[0:64, 2 * dt + 1:2 * dt + 2],
                      ksumb[dt][0:64, 0:1], 0.0)
        nc.scalar.copy(ksum2[64:128, 2 * dt + 1:2 * dt + 2],
                       ksumb[dt][64:128, 0:1])

    # ---------------- Phase V + KV^T accumulation ----------------
    # kvt[c] (128,128) accumulates Vp_pair^T @ Kf_pair over n; diagonal
    # 64-blocks are KV_{2c}^T and KV_{2c+1}^T.
    wv_t = wt.tile([P, KC * DG], MMDT, tag="wt", name="wv_t")
    nc.sync.dma_start(wv_t[:], wv_d[:, :])
    xv_c = []
    for kc in range(KC):
        t = xt.tile([P, L], MMDT, tag="xt", name="xh")
        nc.sync.dma_start(t[:], xv_d[kc * P:(kc + 1) * P, :])
        xv_c.append(t)
    wo_t = wop.tile([P, DT * DM], MMDT, tag="wo", name="wo_t")
    nc.sync.dma_start(wo_t[:], wo_d[:, :])

    # kvc2 off-diagonal zeroing can happen any time (Pool, SBUF-only)
    kvc2 = []
    for c in range(DT):
        kvc = kvsb.tile([P, P], MMDT, tag="kvcat", bufs=4)
        nc.gpsimd.memset(kvc[0:64, 64:128], 0.0)
        nc.gpsimd.memset(kvc[64:128, 0:64], 0.0)
        kvc2.append(kvc)

    kvt = [kvp.tile([P, P], F32, tag="acc", name=f"kvt{_c}")
           for _c in range(DT)]
    vps = []

    def kvt_mm(nt):
        for c in range(DT):
            nc.tensor.matmul(
                kvt[c][:],
                vps[nt][:, c * P:(c + 1) * P],
                kf[nt][:, c * P:(c + 1) * P],
                start=(nt == 0), stop=(nt == NT - 1),
            )

    zbank = None
    zrb = None
    zrA = None
    for nt in range(NT):
        ps = pp.tile([P, 512], F32, tag="pp")
        for kc in range(KC):
            nc.tensor.matmul(
                ps[:],
                xv_c[kc][:, nt * P:(nt + 1) * P],
                wslice(wv_t, kc),
                start=(kc == 0), stop=(kc == KC - 1),
            )
        vp_t = vpp.tile([P, 512], MMDT, tag="vp")
        if nt == NT - 1:
            # split across both PSUM-capable engines: shortest possible
            # latency into the kvt15 -> kvc2 -> M -> Y chain
            nc.scalar.copy(vp_t[:, 0:256], ps[:, 0:256])
            nc.vector.tensor_copy(vp_t[:, 256:512], ps[:, 256:512])
        else:
            nc.scalar.copy(vp_t[:], ps[:])
        vps.append(vp_t)
        # KV^T for nt-1: one tile of slack to cover the vp copy latency
        if nt > 0:
            kvt_mm(nt - 1)
        # Z-prep interleaved into early V so PE never stalls on it
        if nt == 1:
            zbank = pp.tile([P, P], F32, tag="pp", name="zbank")
            idx = 0
            for nt_ in range(NT):
                for dt in range(DT):
                    ccol = nt_ * 8 + dt * 2
                    nc.tensor.matmul(
                        zbank[:, ccol:ccol + 2],
                        qft_block(dt, nt_),
                        ksum2[:, 2 * dt:2 * dt + 2],
                        start=(idx == 0), stop=(idx == NT * DT - 1),
                        skip_group_check=True,
                    )
                    idx += 1
        elif nt == 2:
            zrb = kvsb.tile([P, P], MMDT, tag="zrb", name="zrb")
            with nc.allow_low_precision(reason="zr broadcast is bf16 anyway"):
                nc.vector.reciprocal(zrb[:], zbank[:])
            zrA = kvsb.tile([8, L], MMDT, tag="zrA", name="zrA")
            for nt_ in range(NT):
                ztp = pp.tile([8, P], MMDT, tag="pp", name="ztp")
                nc.tensor.transpose(ztp[:], zrb[:, nt_ * 8:(nt_ + 1) * 8],
                                    ident_b[:])
                nc.vector.tensor_copy(zrA[:, nt_ * P:(nt_ + 1) * P], ztp[:])
        # QfT Z-scaling (exact: KV is block-diagonal per head, so scaling
        # Qf by zr[n, head] == scaling out by zr).  nch handled at nt-4.
        elif 4 <= nt < 8:
            nch = nt - 4
            for dt in range(DT):
                zrp = pp.tile([P, 512], F32, tag="pp", name="zrp")
                nc.tensor.matmul(
                    zrp[:], sel_t[:, dt * P:(dt + 1) * P],
                    zrA[:, nch * 512:(nch + 1) * 512],
                    start=True, stop=True,
                )
                zrs = tmp.tile([P, 512], MMDT, tag="tmp", name="zrs")
                nc.scalar.copy(zrs[:], zrp[:])
                qt = qft[dt * NCH + nch]
                nc.vector.tensor_tensor(qt[:], qt[:], zrs[:], Alu.mult)

    kvt_mm(NT - 1)

    # ---------------- M = blockdiag(KV^T)^T @ Wo rows ----------------
    # m_sb[c] (128 d', 1024) = [M_{2c}; M_{2c+1}] where M_h = KV_h @ Wo_h^T
    # Pipelined per chunk so the first Y matmuls can start early.
    m_sb = []
    for c in range(DT):
        nc.scalar.copy(kvc2[c][0:64, 0:64], kvt[c][0:64, 0:64])
        nc.vector.tensor_copy(kvc2[c][64:128, 64:128], kvt[c][64:128, 64:128])
        mt = msb.tile([P, DM], MMDT, tag="msb", name=f"m{c}")
        for hhalf in range(2):
            mps = pp.tile([P, 512], F32, tag="pp", name="mps")
            nc.tensor.matmul(
                mps[:], kvc2[c][:],
                wo_t[:, c * DM + hhalf * 512:c * DM + (hhalf + 1) * 512],
                start=True, stop=True,
            )
            if hhalf == 0:
                nc.vector.tensor_copy(mt[:, 0:512], mps[:])
            else:
                nc.scalar.copy(mt[:, 512:1024], mps[:])
        m_sb.append(mt)

    # ---------------- Phase Y: y = QfT^T @ M (n-space) ----------------
    for nt in range(NT):
        ypool, ytag = ((kvp, "acc") if nt % 2 == 0 else (pp, "pp"))
        yps0 = ypool.tile([P, 512], F32, tag=ytag, name="yps0")
        yps1 = ypool.tile([P, 512], F32, tag=ytag, name="yps1")
        yt = ysb.tile([P, DM], MMDT, tag="ysb", name="yt")
        # separate c-chains: yps0 finishes 4 matmuls early, so its copy
        # overlaps the yps1 matmuls (shortens the kernel tail)
        for c in range(DT):
            nc.tensor.matmul(
                yps0[:], qft_block(c, nt), m_sb[c][:, 0:512],
                start=(c == 0), stop=(c == DT - 1),
            )
        nc.vector.tensor_copy(yt[:, 0:512], yps0[:])
        for c in range(DT):
            nc.tensor.matmul(
                yps1[:], qft_block(c, nt), m_sb[c][:, 512:1024],
                start=(c == 0), stop=(c == DT - 1),
            )
        if nt < NT - 1:
            nc.scalar.copy(yt[:, 512:1024], yps1[:])
            nc.sync.dma_start(y_d[nt * P:(nt + 1) * P, :], yt[:])
        else:
            # split the last store so the DMA starts as soon as the first
            # half is copied (shorter kernel tail)
            nc.vector.tensor_copy(yt[:, 512:1024], yps1[:])
            nc.sync.dma_start(y_d[nt * P:(nt + 1) * P, 0:512],
                              yt[:, 0:512])
            nc.sync.dma_start(y_d[nt * P:(nt + 1) * P, 512:1024],
                              yt[:, 512:1024])


def make_in_maps(q, k, v, w_q, w_k, w_v, w_o):
    npdt = mybir.dt.np(_mm_dtype())
    q = np.asarray(q, dtype=np.float32)
    k = np.asarray(k, dtype=np.float32)
    v = np.asarray(v, dtype=np.float32)
    w_q = np.asarray(w_q, dtype=np.float32)
    w_k = np.asarray(w_k, dtype=np.float32)
    w_v = np.asarray(w_v, dtype=np.float32)
    w_o = np.asarray(w_o, dtype=np.float32)
    B = q.shape[0]

    def cvt(a):
        return np.ascontiguousarray(a).astype(npdt)

    def wcat(w, g):
        # [DM, DG] slice-transpose -> kc-major [128, KC*DG]
        wT = w[g * DG:(g + 1) * DG, :].T  # [DM, DG]
        return cvt(np.concatenate(
            [wT[kc * P:(kc + 1) * P, :] for kc in range(KC)], axis=1))

    def wocat(w, g):
        # w_o columns for group g -> chunk-major [128, DT*DM]
        woT = w[:, g * DG:(g + 1) * DG].T  # [DG, DM]
        return cvt(np.concatenate(
            [woT[c * P:(c + 1) * P, :] for c in range(DT)], axis=1))

    xqT = [cvt(q[b].T) for b in range(B)]
    xkT = [cvt(k[b].T) for b in range(B)]
    xvT = [cvt(v[b].T) for b in range(B)]
    wqC = [wcat(w_q, g) for g in range(2)]
    wkC = [wcat(w_k, g) for g in range(2)]
    wvC = [wcat(w_v, g) for g in range(2)]
    woC = [wocat(w_o, g) for g in range(2)]
    sel8 = np.zeros((8, DT * P), dtype=np.float32)
    for dt in range(4):
        sel8[2 * dt, dt * P:dt * P + 64] = 1.0
        sel8[2 * dt + 1, dt * P + 64:(dt + 1) * P] = 1.0
    sel8 = sel8.astype(npdt)
    in_maps = []
    for c in range(8):
        b, g = c // 2, c % 2
        in_maps.append({
            "xqT": xqT[b], "xkT": xkT[b], "xvT": xvT[b],
            "wqC": wqC[g], "wkC": wkC[g], "wvC": wvC[g], "woC": woC[g],
            "sel8": sel8,
        })
    return in_maps


def kernel(q, k, v, mask, w_q, w_k, w_v, w_o):
    if "nc" not in _CACHE:
        _CACHE["nc"] = build_nc()
    nc = _CACHE["nc"]
    in_maps = make_in_maps(q, k, v, w_q, w_k, w_v, w_o)
    res = run_bass_kernel_spmd(nc, in_maps, list(range(8)))
    _CACHE["last_results"] = res
    B = np.asarray(q).shape[0]
    out = np.empty((B, L, DM), dtype=np.float32)
    for b in range(B):
        out[b] = (res.results[2 * b]["y"].astype(np.float32)
                  + res.results[2 * b + 1]["y"].astype(np.float32))
    return out
